# revision 7
# baseline (speedup 1.0000x reference)
"""Trainium2 Bass kernel for the sparse-attention (local 3x3 unfold) problem.

Math (per batch-channel (b,c), H=W=128, K=3, pad=1):
  ku = unfold(key)  -> [9, L] raw-flat, reinterpreted [L, 9]
  qu = unfold(query)
  out1 = ku * qu[:, 4:5] ; out2 = ku[:, 4:5] * qu   (as [L, 9] views)

Device layout ("chunked"): per channel a [128, 1152] SBUF tile T where
flat unfold index n = 1152*r + f (r = partition).  Then:
  * chunk view f = 128*s + j: chunk q = 9*r + s equals 128*p + i, i.e. one
    (patch p, image row i) slice of the unfold -> patch loads are <=3 affine
    rect DMAs from a host-padded [130,130] image.
  * group view f = 9*g + e: out[r, g, e] = Tk[r, g, e] * Tq[r, g, 4]
    (uniform stride-9 broadcast multiply, 0-stride e-dim on in1).
The output tile maps to a fully contiguous DRAM range per channel.

Sharding: pure data-parallel over the 256 (b,c) channels; 32 per core.
"""

import sys

for _p in ("/opt/trn_rl_repo", "/opt/pypackages"):
    if _p not in sys.path:
        sys.path.insert(0, _p)

import numpy as np

import concourse.bass as bass
import concourse.mybir as mybir
import concourse.tile as tile
from concourse.bass import AP
from concourse.bass_utils import run_bass_kernel_spmd
from concourse.vector_clock import ScopedClock

# ---------------------------------------------------------------------------
# Patch: this container's walrus rejects >1 sync-wait on the Tile tail Drain
# ("Too many sync wait commands").  Spill extra waits onto SP NOPs, which
# execute in program order before the all-engine barrier, preserving the
# "all work done before sem clear" semantics.
# ---------------------------------------------------------------------------


def _drain_and_barrier(self, tick_clock, wait_clock):
    nc = self.nc
    drain_inst = nc.sync.drain()
    wait_clock.add_sem_waits(
        drain_inst.ins, ScopedClock({None: tick_clock.global_clock})
    )
    si = drain_inst.ins.sync_info
    if si is not None and len(si.on_wait) > 1:
        waits = list(si.on_wait)
        drain_inst.ins.sync_info = mybir.SyncInfo(
            on_wait=waits[:1], on_update=list(si.on_update)
        )
        for w in waits[1:]:
            nop = nc.sync.nop(nofuse=True)
            nop.ins.sync_info = mybir.SyncInfo(on_wait=[w], on_update=[])

    nc.all_engine_barrier()
    assert self.sems is not None
    popped = nc._tile_sem_poison_stack.pop()
    assert popped is self._sem_poison
    nc.clear_and_free_semaphores(list(self.sems.allocated().values()))
    nc.all_engine_barrier()


tile.TileContext._drain_and_barrier = _drain_and_barrier


def _split_waits(nc, maxw=1):
    """Walrus here allows only `maxw` sync-waits per instruction: move extra
    waits onto same-engine NOPs inserted immediately before the instruction
    (same engine stream => executes before it)."""
    for fn in nc.m.functions:
        for bb in fn.blocks:
            out = []
            for inst in bb.instructions:
                si = getattr(inst, "sync_info", None)
                if si is not None and len(si.on_wait) > maxw:
                    waits = list(si.on_wait)
                    for w in waits[:-maxw]:
                        nop = mybir.InstNoOp(
                            name=nc.get_next_instruction_name(),
                            bass_nofuse=True,
                        )
                        nop.engine = inst.engine
                        nop.sync_info = mybir.SyncInfo(on_wait=[w], on_update=[])
                        out.append(nop)
                    inst.sync_info = mybir.SyncInfo(
                        on_wait=waits[-maxw:], on_update=list(si.on_update)
                    )
                out.append(inst)
            bb.instructions[:] = out

# ---------------------------------------------------------------------------

F32 = mybir.dt.float32

N_CORES = 8
B, C, H, W = 4, 64, 128, 128
BC = B * C                # 256 channels
CPC = BC // N_CORES       # 32 channels per core
NCH = 4                   # channels per group (batched in one set of tiles)
NG = CPC // NCH           # groups per core
HP = H + 2                # padded rows
VAR = HP * W              # one dj-variant: [130, 128]
IMG = 3 * VAR             # three dj-variants per channel
L = H * W
CH_FREE = 9 * 128         # 1152 floats per channel per partition
FREE = NCH * CH_FREE      # tile free width
OUT_CH = 9 * L            # 147456 floats per channel output


def _patch_rects(p):
    """Dest rectangles for patch p in (partition a, slot b) space.

    Chunk q = 9*a + b = 128*p + i; returns (alo, ahi, blo, bhi) covering
    i in [0, 128) as <=3 rectangles.
    """
    q0 = 128 * p
    a0, b0 = divmod(q0, 9)
    ae, be = divmod(q0 + 128, 9)
    rects = []
    a_full = a0
    if b0 > 0:
        rects.append((a0, a0 + 1, b0, 9))
        a_full = a0 + 1
    if ae > a_full:
        rects.append((a_full, ae, 0, 9))
    if be > 0:
        rects.append((ae, ae + 1, 0, be))
    return rects


def _build_program():
    nc = bass.Bass(trn_type="TRN2")
    kp = nc.dram_tensor("kp", [CPC, 3, HP, W], F32, kind="ExternalInput")
    qp = nc.dram_tensor("qp", [CPC, 3, HP, W], F32, kind="ExternalInput")
    o1 = nc.dram_tensor("o1", [CPC, OUT_CH], F32, kind="ExternalOutput")
    o2 = nc.dram_tensor("o2", [CPC, OUT_CH], F32, kind="ExternalOutput")

    with tile.TileContext(nc) as tc:
        with (
            tc.tile_pool(name="tin", bufs=2) as tin,
            tc.tile_pool(name="tout", bufs=2) as tout,
        ):
            for g in range(NG):
                tk = tin.tile([128, FREE], F32, tag="tk")
                tq = tin.tile([128, FREE], F32, tag="tq")
                # ---- loads: build chunked unfold tiles ----
                for srcd, t, eng in ((kp, tk, nc.sync), (qp, tq, nc.scalar)):
                    th = t[:].tensor
                    for p in range(9):
                        di, dj = divmod(p, 3)
                        q0 = 128 * p
                        for (alo, ahi, blo, bhi) in _patch_rects(p):
                            na, nb = ahi - alo, bhi - blo
                            dst = AP(
                                th,
                                alo * FREE + blo * 128,
                                [[FREE, na], [CH_FREE, NCH], [1, nb * W]],
                            )
                            i0 = 9 * alo + blo - q0
                            src = AP(
                                srcd,
                                g * NCH * IMG + dj * VAR + (i0 + di) * W,
                                [[9 * W, na], [IMG, NCH], [1, nb * W]],
                            )
                            eng.dma_start(dst, src)

                # ---- multiply: uniform stride-9 center broadcast ----
                o1t = tout.tile([128, FREE], F32, tag="o1t")
                o2t = tout.tile([128, FREE], F32, tag="o2t")
                tkh, tqh = tk[:].tensor, tq[:].tensor
                for ch in range(NCH):
                    base = ch * CH_FREE
                    n_ap = [[FREE, 128], [9, 128], [1, 9]]
                    b_ap = [[FREE, 128], [9, 128], [0, 9]]
                    nc.vector.tensor_mul(
                        AP(o1t[:].tensor, base, n_ap),
                        AP(tkh, base, n_ap),
                        AP(tqh, base + 4, b_ap),
                    )
                    nc.vector.tensor_mul(
                        AP(o2t[:].tensor, base, n_ap),
                        AP(tqh, base, n_ap),
                        AP(tkh, base + 4, b_ap),
                    )

                # ---- stores: contiguous per channel ----
                for od, ot, eng in ((o1, o1t, nc.sync), (o2, o2t, nc.scalar)):
                    src = AP(
                        ot[:].tensor,
                        0,
                        [[FREE, 128], [CH_FREE, NCH], [1, CH_FREE]],
                    )
                    dst = AP(
                        od,
                        g * NCH * OUT_CH,
                        [[CH_FREE, 128], [OUT_CH, NCH], [1, CH_FREE]],
                    )
                    eng.dma_start(dst, src)
    _split_waits(nc)
    return nc


_NC_CACHE = []


def _get_nc():
    if not _NC_CACHE:
        _NC_CACHE.append(_build_program())
    return _NC_CACHE[0]


def _variants(x):
    """[BC,H,W] -> [BC, 3, HP, W]: dj-shifted, row-padded column windows."""
    xpad = np.pad(
        np.ascontiguousarray(x, dtype=np.float32).reshape(BC, H, W),
        ((0, 0), (1, 1), (1, 1)),
    )
    return np.stack([xpad[:, :, v : v + W] for v in range(3)], axis=1)


def make_in_maps(key_map, query_map):
    kv = _variants(key_map)
    qv = _variants(query_map)
    return [
        {
            "kp": kv[m * CPC : (m + 1) * CPC],
            "qp": qv[m * CPC : (m + 1) * CPC],
        }
        for m in range(N_CORES)
    ]


def assemble(results):
    out1 = np.concatenate([results[m]["o1"] for m in range(N_CORES)], axis=0)
    out2 = np.concatenate([results[m]["o2"] for m in range(N_CORES)], axis=0)
    return (
        out1.reshape(B, C, L, 9),
        out2.reshape(B, C, L, 9),
    )


def kernel(key_map, query_map):
    nc = _get_nc()
    in_maps = make_in_maps(key_map, query_map)
    res = run_bass_kernel_spmd(nc, in_maps, core_ids=list(range(N_CORES)))
    return assemble(res.results)


# revision 9
# speedup vs baseline: 1.0434x; 1.0434x over previous
"""Trainium2 Bass kernel for the sparse-attention (local 3x3 unfold) problem.

Math (per batch-channel (b,c), H=W=128, K=3, pad=1):
  ku = unfold(key)  -> [9, L] raw-flat, reinterpreted [L, 9]
  qu = unfold(query)
  out1 = ku * qu[:, 4:5] ; out2 = ku[:, 4:5] * qu   (as [L, 9] views)

Device layout ("chunked"): per channel a [128, 1152] SBUF tile T where
flat unfold index n = 1152*r + f (r = partition).  Then:
  * chunk view f = 128*s + j: chunk q = 9*r + s equals 128*p + i, i.e. one
    (patch p, image row i) slice of the unfold -> patch loads are <=3 affine
    rect DMAs from a host-padded [130,130] image.
  * group view f = 9*g + e: out[r, g, e] = Tk[r, g, e] * Tq[r, g, 4]
    (uniform stride-9 broadcast multiply, 0-stride e-dim on in1).
The output tile maps to a fully contiguous DRAM range per channel.

Sharding: pure data-parallel over the 256 (b,c) channels; 32 per core.
"""

import sys

for _p in ("/opt/trn_rl_repo", "/opt/pypackages"):
    if _p not in sys.path:
        sys.path.insert(0, _p)

import numpy as np

import concourse.bass as bass
import concourse.mybir as mybir
import concourse.tile as tile
from concourse.bass import AP
from concourse.bass_utils import run_bass_kernel_spmd
from concourse.vector_clock import ScopedClock

# ---------------------------------------------------------------------------
# Patch: this container's walrus rejects >1 sync-wait on the Tile tail Drain
# ("Too many sync wait commands").  Spill extra waits onto SP NOPs, which
# execute in program order before the all-engine barrier, preserving the
# "all work done before sem clear" semantics.
# ---------------------------------------------------------------------------


def _drain_and_barrier(self, tick_clock, wait_clock):
    nc = self.nc
    drain_inst = nc.sync.drain()
    wait_clock.add_sem_waits(
        drain_inst.ins, ScopedClock({None: tick_clock.global_clock})
    )
    si = drain_inst.ins.sync_info
    if si is not None and len(si.on_wait) > 1:
        waits = list(si.on_wait)
        drain_inst.ins.sync_info = mybir.SyncInfo(
            on_wait=waits[:1], on_update=list(si.on_update)
        )
        for w in waits[1:]:
            nop = nc.sync.nop(nofuse=True)
            nop.ins.sync_info = mybir.SyncInfo(on_wait=[w], on_update=[])

    nc.all_engine_barrier()
    assert self.sems is not None
    popped = nc._tile_sem_poison_stack.pop()
    assert popped is self._sem_poison
    nc.clear_and_free_semaphores(list(self.sems.allocated().values()))
    nc.all_engine_barrier()


tile.TileContext._drain_and_barrier = _drain_and_barrier


def _split_waits(nc, maxw=1):
    """Walrus here allows only `maxw` sync-waits per instruction: move extra
    waits onto same-engine NOPs inserted immediately before the instruction
    (same engine stream => executes before it)."""
    for fn in nc.m.functions:
        for bb in fn.blocks:
            out = []
            for inst in bb.instructions:
                si = getattr(inst, "sync_info", None)
                if si is not None and len(si.on_wait) > maxw:
                    waits = list(si.on_wait)
                    for w in waits[:-maxw]:
                        nop = mybir.InstNoOp(
                            name=nc.get_next_instruction_name(),
                            bass_nofuse=True,
                        )
                        nop.engine = inst.engine
                        nop.sync_info = mybir.SyncInfo(on_wait=[w], on_update=[])
                        out.append(nop)
                    inst.sync_info = mybir.SyncInfo(
                        on_wait=waits[-maxw:], on_update=list(si.on_update)
                    )
                out.append(inst)
            bb.instructions[:] = out

# ---------------------------------------------------------------------------

F32 = mybir.dt.float32

N_CORES = 8
B, C, H, W = 4, 64, 128, 128
BC = B * C                # 256 channels
CPC = BC // N_CORES       # 32 channels per core
NCH = 8                   # channels per input group (one set of load tiles)
NCO = 2                   # channels per output tile (SBUF budget)
NG = CPC // NCH           # input groups per core
HP = H + 2                # padded rows
VAR = HP * W              # one dj-variant: [130, 128]
IMG = 3 * VAR             # three dj-variants per channel
L = H * W
CH_FREE = 9 * 128         # 1152 floats per channel per partition
FREE = NCH * CH_FREE      # input tile free width
OFREE = NCO * CH_FREE     # output tile free width
OUT_CH = 9 * L            # 147456 floats per channel output


def _patch_rects(p):
    """Dest rectangles for patch p in (partition a, slot b) space.

    Chunk q = 9*a + b = 128*p + i; returns (alo, ahi, blo, bhi) covering
    i in [0, 128) as <=3 rectangles.
    """
    q0 = 128 * p
    a0, b0 = divmod(q0, 9)
    ae, be = divmod(q0 + 128, 9)
    rects = []
    a_full = a0
    if b0 > 0:
        rects.append((a0, a0 + 1, b0, 9))
        a_full = a0 + 1
    if ae > a_full:
        rects.append((a_full, ae, 0, 9))
    if be > 0:
        rects.append((ae, ae + 1, 0, be))
    return rects


def _build_program():
    nc = bass.Bass(trn_type="TRN2")
    kp = nc.dram_tensor("kp", [CPC, 3, HP, W], F32, kind="ExternalInput")
    qp = nc.dram_tensor("qp", [CPC, 3, HP, W], F32, kind="ExternalInput")
    o1 = nc.dram_tensor("o1", [CPC, OUT_CH], F32, kind="ExternalOutput")
    o2 = nc.dram_tensor("o2", [CPC, OUT_CH], F32, kind="ExternalOutput")

    # Load-DMA issue engines: HWDGE (~0.63us on shared HWDGE device via
    # SP/ACT) vs SWDGE (~1us on otherwise-idle Pool SEQ).  Interleave so
    # neither descriptor-generation path becomes the bottleneck.
    load_engines = [nc.sync, nc.gpsimd, nc.scalar, nc.gpsimd]

    with tile.TileContext(nc) as tc:
        with (
            tc.tile_pool(name="tin", bufs=2) as tin,
            tc.tile_pool(name="tout", bufs=2) as tout,
        ):
            eng_i = 0
            for g in range(NG):
                tk = tin.tile([128, FREE], F32, tag="tk")
                tq = tin.tile([128, FREE], F32, tag="tq")
                # ---- loads: build chunked unfold tiles ----
                for srcd, t in ((kp, tk), (qp, tq)):
                    th = t[:].tensor
                    for p in range(9):
                        di, dj = divmod(p, 3)
                        q0 = 128 * p
                        for (alo, ahi, blo, bhi) in _patch_rects(p):
                            na, nb = ahi - alo, bhi - blo
                            dst = AP(
                                th,
                                alo * FREE + blo * 128,
                                [[FREE, na], [CH_FREE, NCH], [1, nb * W]],
                            )
                            i0 = 9 * alo + blo - q0
                            src = AP(
                                srcd,
                                g * NCH * IMG + dj * VAR + (i0 + di) * W,
                                [[9 * W, na], [IMG, NCH], [1, nb * W]],
                            )
                            load_engines[eng_i % len(load_engines)].dma_start(
                                dst, src
                            )
                            eng_i += 1

                tkh, tqh = tk[:].tensor, tq[:].tensor
                for og in range(NCH // NCO):
                    # ---- multiply: uniform stride-9 center broadcast ----
                    o1t = tout.tile([128, OFREE], F32, tag="o1t")
                    o2t = tout.tile([128, OFREE], F32, tag="o2t")
                    for ch in range(NCO):
                        ibase = (og * NCO + ch) * CH_FREE
                        obase = ch * CH_FREE
                        in_ap = [[FREE, 128], [9, 128], [1, 9]]
                        bc_ap = [[FREE, 128], [9, 128], [0, 9]]
                        o_ap = [[OFREE, 128], [9, 128], [1, 9]]
                        nc.vector.tensor_mul(
                            AP(o1t[:].tensor, obase, o_ap),
                            AP(tkh, ibase, in_ap),
                            AP(tqh, ibase + 4, bc_ap),
                        )
                        nc.vector.tensor_mul(
                            AP(o2t[:].tensor, obase, o_ap),
                            AP(tqh, ibase, in_ap),
                            AP(tkh, ibase + 4, bc_ap),
                        )

                    # ---- stores: contiguous per channel ----
                    for od, ot, eng in ((o1, o1t, nc.sync), (o2, o2t, nc.scalar)):
                        src = AP(
                            ot[:].tensor,
                            0,
                            [[OFREE, 128], [CH_FREE, NCO], [1, CH_FREE]],
                        )
                        dst = AP(
                            od,
                            (g * NCH + og * NCO) * OUT_CH,
                            [[CH_FREE, 128], [OUT_CH, NCO], [1, CH_FREE]],
                        )
                        eng.dma_start(dst, src)
    _split_waits(nc)
    return nc


_NC_CACHE = []


def _get_nc():
    if not _NC_CACHE:
        _NC_CACHE.append(_build_program())
    return _NC_CACHE[0]


def _variants(x):
    """[BC,H,W] -> [BC, 3, HP, W]: dj-shifted, row-padded column windows."""
    xpad = np.pad(
        np.ascontiguousarray(x, dtype=np.float32).reshape(BC, H, W),
        ((0, 0), (1, 1), (1, 1)),
    )
    return np.stack([xpad[:, :, v : v + W] for v in range(3)], axis=1)


def make_in_maps(key_map, query_map):
    kv = _variants(key_map)
    qv = _variants(query_map)
    return [
        {
            "kp": kv[m * CPC : (m + 1) * CPC],
            "qp": qv[m * CPC : (m + 1) * CPC],
        }
        for m in range(N_CORES)
    ]


def assemble(results):
    out1 = np.concatenate([results[m]["o1"] for m in range(N_CORES)], axis=0)
    out2 = np.concatenate([results[m]["o2"] for m in range(N_CORES)], axis=0)
    return (
        out1.reshape(B, C, L, 9),
        out2.reshape(B, C, L, 9),
    )


def kernel(key_map, query_map):
    nc = _get_nc()
    in_maps = make_in_maps(key_map, query_map)
    res = run_bass_kernel_spmd(nc, in_maps, core_ids=list(range(N_CORES)))
    return assemble(res.results)


# revision 12
# speedup vs baseline: 90.5780x; 86.8135x over previous
"""Trainium2 Bass kernel for the sparse-attention (local 3x3 unfold) problem.

Math (per batch-channel (b,c), H=W=128, K=3, pad=1):
  ku = unfold(key)  -> [9, L] raw-flat, reinterpreted [L, 9]
  qu = unfold(query)
  out1 = ku * qu[:, 4:5] ; out2 = ku[:, 4:5] * qu   (as [L, 9] views)

Device layout ("chunked"): per channel a [128, 1152] SBUF tile T where
flat unfold index n = 1152*r + f (r = partition).  Then:
  * chunk view f = 128*s + j: chunk q = 9*r + s equals 128*p + i, i.e. one
    (patch p, image row i) slice of the unfold -> patch loads are <=3 affine
    rect DMAs from a host-padded [130,130] image.
  * group view f = 9*g + e: out[r, g, e] = Tk[r, g, e] * Tq[r, g, 4]
    (uniform stride-9 broadcast multiply, 0-stride e-dim on in1).
The output tile maps to a fully contiguous DRAM range per channel.

Sharding: pure data-parallel over the 256 (b,c) channels; 32 per core.
"""

import sys

for _p in ("/opt/trn_rl_repo", "/opt/pypackages"):
    if _p not in sys.path:
        sys.path.insert(0, _p)

import numpy as np

import concourse.bass as bass
import concourse.mybir as mybir
import concourse.tile as tile
from concourse.bass import AP
from concourse.bass_utils import run_bass_kernel_spmd
from concourse.vector_clock import ScopedClock

# ---------------------------------------------------------------------------
# Patch: this container's walrus rejects >1 sync-wait on the Tile tail Drain
# ("Too many sync wait commands").  Spill extra waits onto SP NOPs, which
# execute in program order before the all-engine barrier, preserving the
# "all work done before sem clear" semantics.
# ---------------------------------------------------------------------------


def _drain_and_barrier(self, tick_clock, wait_clock):
    nc = self.nc
    drain_inst = nc.sync.drain()
    wait_clock.add_sem_waits(
        drain_inst.ins, ScopedClock({None: tick_clock.global_clock})
    )
    si = drain_inst.ins.sync_info
    if si is not None and len(si.on_wait) > 1:
        waits = list(si.on_wait)
        drain_inst.ins.sync_info = mybir.SyncInfo(
            on_wait=waits[:1], on_update=list(si.on_update)
        )
        for w in waits[1:]:
            nop = nc.sync.nop(nofuse=True)
            nop.ins.sync_info = mybir.SyncInfo(on_wait=[w], on_update=[])

    nc.all_engine_barrier()
    assert self.sems is not None
    popped = nc._tile_sem_poison_stack.pop()
    assert popped is self._sem_poison
    nc.clear_and_free_semaphores(list(self.sems.allocated().values()))
    nc.all_engine_barrier()


tile.TileContext._drain_and_barrier = _drain_and_barrier


def _split_waits(nc, maxw=1):
    """Walrus here allows only `maxw` sync-waits per instruction: move extra
    waits onto same-engine NOPs inserted immediately before the instruction
    (same engine stream => executes before it)."""
    for fn in nc.m.functions:
        for bb in fn.blocks:
            out = []
            for inst in bb.instructions:
                si = getattr(inst, "sync_info", None)
                if si is not None and len(si.on_wait) > maxw:
                    waits = list(si.on_wait)
                    for w in waits[:-maxw]:
                        nop = mybir.InstNoOp(
                            name=nc.get_next_instruction_name(),
                            bass_nofuse=True,
                        )
                        nop.engine = inst.engine
                        nop.sync_info = mybir.SyncInfo(on_wait=[w], on_update=[])
                        nc.register_instruction(nop)
                        out.append(nop)
                    inst.sync_info = mybir.SyncInfo(
                        on_wait=waits[-maxw:], on_update=list(si.on_update)
                    )
                out.append(inst)
            bb.instructions[:] = out

# ---------------------------------------------------------------------------

F32 = mybir.dt.float32

N_CORES = 8
B, C, H, W = 4, 64, 128, 128
BC = B * C                # 256 channels
CPC = BC // N_CORES       # 32 channels per core
NCH = 8                   # channels per input group (one set of load tiles)
NCO = 2                   # channels per output tile (SBUF budget)
NG = CPC // NCH           # input groups per core
HP = H + 2                # padded rows
VAR = HP * W              # one dj-variant: [130, 128]
IMG = 3 * VAR             # three dj-variants per channel
L = H * W
CH_FREE = 9 * 128         # 1152 floats per channel per partition
FREE = NCH * CH_FREE      # input tile free width
OFREE = NCO * CH_FREE     # output tile free width
OUT_CH = 9 * L            # 147456 floats per channel output


def _patch_rects(p):
    """Dest rectangles for patch p in (partition a, slot b) space.

    Chunk q = 9*a + b = 128*p + i; returns (alo, ahi, blo, bhi) covering
    i in [0, 128) as <=3 rectangles.
    """
    q0 = 128 * p
    a0, b0 = divmod(q0, 9)
    ae, be = divmod(q0 + 128, 9)
    rects = []
    a_full = a0
    if b0 > 0:
        rects.append((a0, a0 + 1, b0, 9))
        a_full = a0 + 1
    if ae > a_full:
        rects.append((a_full, ae, 0, 9))
    if be > 0:
        rects.append((ae, ae + 1, 0, be))
    return rects


def _build_program(reps=1):
    nc = bass.Bass(trn_type="TRN2")
    kp = nc.dram_tensor("kp", [CPC, 3, HP, W], F32, kind="ExternalInput")
    qp = nc.dram_tensor("qp", [CPC, 3, HP, W], F32, kind="ExternalInput")
    o1 = nc.dram_tensor("o1", [CPC, OUT_CH], F32, kind="ExternalOutput")
    o2 = nc.dram_tensor("o2", [CPC, OUT_CH], F32, kind="ExternalOutput")

    # Load-DMA issue engines: HWDGE (~0.63us on shared HWDGE device via
    # SP/ACT) vs SWDGE (~1us on otherwise-idle Pool SEQ).  Interleave so
    # neither descriptor-generation path becomes the bottleneck.
    load_engines = [nc.sync, nc.gpsimd, nc.scalar, nc.gpsimd]

    with tile.TileContext(nc) as tc:
        with (
            tc.tile_pool(name="tin", bufs=2) as tin,
            tc.tile_pool(name="tout", bufs=2) as tout,
        ):
            eng_i = 0
            for g in [g for _ in range(reps) for g in range(NG)]:
                tk = tin.tile([128, FREE], F32, tag="tk")
                tq = tin.tile([128, FREE], F32, tag="tq")
                # ---- loads: build chunked unfold tiles ----
                for srcd, t in ((kp, tk), (qp, tq)):
                    th = t[:].tensor
                    for p in range(9):
                        di, dj = divmod(p, 3)
                        q0 = 128 * p
                        for (alo, ahi, blo, bhi) in _patch_rects(p):
                            na, nb = ahi - alo, bhi - blo
                            dst = AP(
                                th,
                                alo * FREE + blo * 128,
                                [[FREE, na], [CH_FREE, NCH], [1, nb * W]],
                            )
                            i0 = 9 * alo + blo - q0
                            src = AP(
                                srcd,
                                g * NCH * IMG + dj * VAR + (i0 + di) * W,
                                [[9 * W, na], [IMG, NCH], [1, nb * W]],
                            )
                            load_engines[eng_i % len(load_engines)].dma_start(
                                dst, src
                            )
                            eng_i += 1

                tkh, tqh = tk[:].tensor, tq[:].tensor
                for og in range(NCH // NCO):
                    # ---- multiply: uniform stride-9 center broadcast ----
                    o1t = tout.tile([128, OFREE], F32, tag="o1t")
                    o2t = tout.tile([128, OFREE], F32, tag="o2t")
                    for ch in range(NCO):
                        ibase = (og * NCO + ch) * CH_FREE
                        obase = ch * CH_FREE
                        in_ap = [[FREE, 128], [9, 128], [1, 9]]
                        bc_ap = [[FREE, 128], [9, 128], [0, 9]]
                        o_ap = [[OFREE, 128], [9, 128], [1, 9]]
                        nc.vector.tensor_mul(
                            AP(o1t[:].tensor, obase, o_ap),
                            AP(tkh, ibase, in_ap),
                            AP(tqh, ibase + 4, bc_ap),
                        )
                        nc.vector.tensor_mul(
                            AP(o2t[:].tensor, obase, o_ap),
                            AP(tqh, ibase, in_ap),
                            AP(tkh, ibase + 4, bc_ap),
                        )

                    # ---- stores: contiguous per channel ----
                    for od, ot, eng in ((o1, o1t, nc.sync), (o2, o2t, nc.scalar)):
                        src = AP(
                            ot[:].tensor,
                            0,
                            [[OFREE, 128], [CH_FREE, NCO], [1, CH_FREE]],
                        )
                        dst = AP(
                            od,
                            (g * NCH + og * NCO) * OUT_CH,
                            [[CH_FREE, 128], [OUT_CH, NCO], [1, CH_FREE]],
                        )
                        eng.dma_start(dst, src)
    _split_waits(nc)
    return nc


_NC_CACHE = []


def _get_nc():
    if not _NC_CACHE:
        _NC_CACHE.append(_build_program())
    return _NC_CACHE[0]


def _variants(x):
    """[BC,H,W] -> [BC, 3, HP, W]: dj-shifted, row-padded column windows."""
    xpad = np.pad(
        np.ascontiguousarray(x, dtype=np.float32).reshape(BC, H, W),
        ((0, 0), (1, 1), (1, 1)),
    )
    return np.stack([xpad[:, :, v : v + W] for v in range(3)], axis=1)


def make_in_maps(key_map, query_map):
    kv = _variants(key_map)
    qv = _variants(query_map)
    return [
        {
            "kp": kv[m * CPC : (m + 1) * CPC],
            "qp": qv[m * CPC : (m + 1) * CPC],
        }
        for m in range(N_CORES)
    ]


def assemble(results):
    out1 = np.concatenate([results[m]["o1"] for m in range(N_CORES)], axis=0)
    out2 = np.concatenate([results[m]["o2"] for m in range(N_CORES)], axis=0)
    return (
        out1.reshape(B, C, L, 9),
        out2.reshape(B, C, L, 9),
    )


def kernel(key_map, query_map):
    nc = _get_nc()
    in_maps = make_in_maps(key_map, query_map)
    res = run_bass_kernel_spmd(nc, in_maps, core_ids=list(range(N_CORES)))
    return assemble(res.results)
